# revision 40
# baseline (speedup 1.0000x reference)
"""ClusterLoss Bass/Tile kernel for Trainium2 (8 NeuronCores, data parallel).

Strategy (v2: TensorE dot products, ragged DMA)
-----------------------------------------------
Samples are globally sorted by hn_count (descending) and stripe-dealt to the
8 cores, so each core's in-core rank k holds a sample whose negative count is
nearly identical across cores.  Pairs of ranks share a compile-time negative
bound N_g, making per-pair DMA nearly exactly the valid (ragged) data.

All dot products (q.hn, q.fn, q.k, q.k2) run on the Tensor engine: for each
sample, a [128, 32] weight tile holding the sample's qT d-chunk in column
(rank%32) and zeros elsewhere is multiplied against the raw transposed
hn/fn/k/k2 columns ([d', W] bf16, streamed at 1 col/cycle), accumulating into
psum rows [32j, 32j+32) via column tiling.  After 128 samples, the psum bank
holds lneg[sample, n] plus fn dots and the k/k2 dots; ScalarE copies it out
with the 1/TEMP scale folded in.  The DVE only builds the tiny q-column
weights and runs the masked-logsumexp / cross-entropy / BML epilogue, so the
kernel is DMA-bound rather than vector-bound.

Per-block psum layout (columns): [0,32) fn dots | 32 q.k | 33 q.k2 |
[34, 34+N) hn dots.  Rows = samples.  The psum bank is zeroed by the DVE
once per block and every matmul accumulates (start=False).

The program is JIT-specialized to the per-pair bounds tuple derived from the
counts at call time and cached, so repeated calls with the same raggedness
profile reuse the compiled NEFF.
"""

from contextlib import ExitStack

import numpy as np

import concourse.bass as bass
import concourse.bacc as bacc
import concourse.tile as tile
from concourse import mybir
from concourse import bass_utils

N_CORES = 8
B, D, N_MAX, M_MAX = 2048, 512, 256, 32
B_LOC = B // N_CORES          # 256 samples per core
PBLK = 128                    # partition block (samples per psum bank)
PRE = M_MAX + 2               # fn cols + k + k2 before the hn columns
GRP = 8                       # samples per DMA group (uniform W within group)
NGRP = B_LOC // GRP           # 32 groups per core
HBUFS = 9                     # group-tile ring

TEMP, ALPHA, BETA, LAMBDA_BML = 0.07, 0.4, 0.2, 0.2
NEG = -1e30
EXP_CLAMP = -87.0

F32 = mybir.dt.float32
F16 = mybir.dt.float16
BF16 = mybir.dt.bfloat16
I32 = mybir.dt.int32
AF = mybir.ActivationFunctionType
OP = mybir.AluOpType
AX = mybir.AxisListType


def _col_ap(qw, r):
    """Column r of a [128, 4, 32] tile: 4 elements with stride 32."""
    a = qw[:]
    return bass.AP(tensor=a.tensor, offset=a.offset + r, ap=[a.ap[0], [32, 4]])


def _emit(tc, Ns, hp, qT, hc, fc, out):
    nc = tc.nc
    NB = (max(Ns[0], 1), max(Ns[NGRP // 2], 1))
    with ExitStack() as ctx:
        hpool = ctx.enter_context(tc.tile_pool(name="hp", bufs=HBUFS))
        qpool = ctx.enter_context(tc.tile_pool(name="qp", bufs=1))
        mpool = ctx.enter_context(tc.tile_pool(name="med", bufs=2))
        smpool = ctx.enter_context(tc.tile_pool(name="sm", bufs=2))
        cpool = ctx.enter_context(tc.tile_pool(name="cst", bufs=1))
        ppool = ctx.enter_context(tc.tile_pool(name="ps", bufs=2, space="PSUM"))
        fpool = ctx.enter_context(tc.tile_pool(name="fin", bufs=1, space="PSUM"))

        # constants
        iota_i = cpool.tile([PBLK, N_MAX], I32, tag="iota_i", name="iota_i")
        nc.gpsimd.iota(iota_i[:], pattern=[[1, N_MAX]], base=0, channel_multiplier=0)
        iota_f = cpool.tile([PBLK, N_MAX], F32, tag="iota_f", name="iota_f")
        nc.vector.tensor_copy(out=iota_f[:], in_=iota_i[:])
        ones = cpool.tile([PBLK, 1], F32, tag="ones", name="ones")
        nc.vector.memset(ones[:], 1.0)
        alpha_t = cpool.tile([PBLK, 1], F32, tag="alpha_t", name="alpha_t")
        nc.vector.memset(alpha_t[:], ALPHA)
        nbeta_t = cpool.tile([PBLK, 1], F32, tag="nbeta_t", name="nbeta_t")
        nc.vector.memset(nbeta_t[:], -BETA)

        qT_t = qpool.tile([PBLK, B_LOC, 4], BF16, tag="qT", name="qT_t")
        nc.sync.dma_start(out=qT_t[:], in_=qT[:])

        # q-column weight tiles: per (block, strip, parity), zeroed once.
        # block 1 needs 4 parities (two of its groups run concurrently).
        qw = {}
        qw_last = {}
        for b in range(2):
            for j in range(4):
                for par in range(4):
                    t = cpool.tile([PBLK, 4, 32], BF16, tag=f"qw{b}{j}{par}",
                                   name=f"qw{b}{j}{par}")
                    nc.vector.memset(t[:], 0.0)
                    qw[(b, j, par)] = t
                    qw_last[(b, j, par)] = None

        def sm(tagname, dt=F32, w=1):
            return smpool.tile([PBLK, w], dt, tag=tagname, name=tagname)

        hp_ap = hp[:]
        sizes = [GRP * 4 * PBLK * (PRE + N) for N in Ns]
        goffs = [0]
        for sz in sizes:
            goffs.append(goffs[-1] + sz)

        HALF = NGRP // 2
        ps_t = []
        for b in range(2):
            ps = ppool.tile([PBLK, 512], F32, tag="ps", name=f"ps{b}")
            nc.vector.memset(ps[:], 0.0)
            ps_t.append(ps)

        # Warm every activation table set used by the epilogue now, so no
        # ACT_TABLE_LOAD lands on the post-matmul critical path.
        warm = sm("warm")
        nc.scalar.activation(out=warm[:], in_=ones[:], func=AF.Exp)
        nc.scalar.activation(out=warm[:], in_=warm[:], func=AF.Ln)
        nc.scalar.activation(out=warm[:], in_=warm[:], func=AF.Relu,
                             bias=alpha_t[:], scale=1.0)
        nc.scalar.activation(out=warm[:], in_=warm[:], func=AF.Copy,
                             scale=1.0)

        def load_counts():
            hc_f_t, fc_f_t = [], []
            for b in range(2):
                hc_i = sm(f"hc_i{b}", I32)
                nc.sync.dma_start(out=hc_i[:], in_=hc[b])
                fc_i = sm(f"fc_i{b}", I32)
                nc.sync.dma_start(out=fc_i[:], in_=fc[b])
                hc_f = sm(f"hc_f{b}")
                nc.vector.tensor_copy(out=hc_f[:], in_=hc_i[:])
                fc_f = sm(f"fc_f{b}")
                nc.vector.tensor_copy(out=fc_f[:], in_=fc_i[:])
                hc_f_t.append(hc_f)
                fc_f_t.append(fc_f)
            return hc_f_t, fc_f_t

        # Each phase pairs a big block-0 group with a small block-1 group
        # whose strip differs, so consecutive matmuls hit different
        # col-groups (array-concurrent) and stream-bound / issue-bound work
        # mixes evenly over the timeline.
        def emit_phase(phase):
            entries = [(0, phase, 0), (1, HALF + (phase + 8) % HALF, 1)]
            work = []            # (b, j, r0, W, tile, parbase, last_mm)
            for (b, g, pslot) in entries:
                N = Ns[g]
                W = PRE + N
                gl = g - b * HALF
                gps = 32 // GRP      # groups per strip
                j, r0 = gl // gps, (gl % gps) * GRP
                t = hpool.tile([PBLK, GRP, 4, W], BF16, tag="hpt",
                               name=f"hp{g}")
                # host layout is partition-major: one contiguous GRP*4*W run
                # per partition -> 128 DMA descriptors per group
                src = bass.AP(
                    tensor=hp_ap.tensor,
                    offset=goffs[g],
                    ap=[[GRP * 4 * W, PBLK], [4 * W, GRP], [W, 4], [1, W]],
                )
                nc.sync.dma_start(out=t[:], in_=src)
                lastg = (phase == HALF - 1)
                parbase = 0
                work.append((b, j, r0, W, t, parbase, lastg))
            for i in range(GRP):
                for (b, j, r0, W, t, parbase, lastg) in work:
                    r = r0 + i
                    s = PBLK * b + 32 * j + r
                    key = (b, j, parbase + (i % 4))
                    w_t = qw[key]
                    if qw_last[key] is not None:
                        nc.vector.memset(_col_ap(w_t, qw_last[key]), 0.0)
                    qw_last[key] = r
                    nc.vector.tensor_copy(out=_col_ap(w_t, r),
                                          in_=qT_t[:, s, :])
                for dc in range(4):
                    for (b, j, r0, W, t, parbase, lastg) in work:
                        w_t = qw[(b, j, parbase + (i % 4))]
                        stop = (lastg and i == GRP - 1 and dc == 3)
                        nc.tensor.matmul(
                            ps_t[b][32 * j:32 * j + 32, 0:W],
                            lhsT=w_t[:, dc, :],
                            rhs=t[:, i, dc, :],
                            start=False,
                            stop=stop,
                            tile_position=(0, 32 * j),
                            skip_group_check=True,
                        )

        blk_contribs = [None, None]

        def epilogue_both():
            """Both blocks' epilogues, op-interleaved so neither chain
            head-of-line-blocks the other on the FIFO engine queues."""
            S = [dict(ps=ps_t[b], hcf=hc_f_t[b], fcf=fc_f_t[b], NBb=NB[b])
                 for b in range(2)]

            def T(b, tagname, w=1):
                return smpool.tile([PBLK, w], F32, tag=f"{tagname}{b}",
                                   name=f"{tagname}{b}")

            for b in (0, 1):
                s = S[b]
                s["lneg"] = mpool.tile([PBLK, s["NBb"]], F32, tag=f"lneg{b}",
                                       name=f"lneg{b}")
                nc.scalar.activation(out=s["lneg"][:],
                                     in_=s["ps"][:, PRE:PRE + s["NBb"]],
                                     func=AF.Copy, scale=1.0 / TEMP)
            for b in (0, 1):
                s = S[b]
                s["fnd"] = T(b, "fnd", w=M_MAX)
                nc.scalar.activation(out=s["fnd"][:], in_=s["ps"][:, 0:M_MAX],
                                     func=AF.Copy, scale=1.0)
                s["lp2"] = T(b, "lp2", w=2)
                nc.scalar.activation(out=s["lp2"][:], in_=s["ps"][:, 32:34],
                                     func=AF.Copy, scale=1.0 / TEMP)
            for b in (0, 1):
                s = S[b]
                s["mneg"] = mpool.tile([PBLK, s["NBb"]], F32, tag=f"mneg{b}",
                                       name=f"mneg{b}")
                nc.vector.tensor_scalar(
                    out=s["mneg"][:], in0=iota_f[:, :s["NBb"]],
                    scalar1=s["hcf"][:], scalar2=NEG,
                    op0=OP.is_ge, op1=OP.mult)
            for b in (0, 1):
                s = S[b]
                nc.vector.tensor_add(out=s["lneg"][:], in0=s["lneg"][:],
                                     in1=s["mneg"][:])
            for b in (0, 1):
                s = S[b]
                s["mrow"] = T(b, "mrow")
                nc.vector.tensor_reduce(out=s["mrow"][:], in_=s["lneg"][:],
                                        axis=AX.X, op=OP.max)
            for b in (0, 1):
                s = S[b]
                s["expin"] = mpool.tile([PBLK, s["NBb"]], F32, tag=f"expin{b}",
                                        name=f"expin{b}")
                nc.vector.tensor_scalar(
                    out=s["expin"][:], in0=s["lneg"][:], scalar1=s["mrow"][:],
                    scalar2=EXP_CLAMP, op0=OP.subtract, op1=OP.max)
            for b in (0, 1):
                s = S[b]
                expout = mpool.tile([PBLK, s["NBb"]], F32, tag=f"expout{b}",
                                    name=f"expout{b}")
                s["sumexp"] = T(b, "sumexp")
                nc.scalar.activation(out=expout[:], in_=s["expin"][:],
                                     func=AF.Exp, accum_out=s["sumexp"][:])
            for b in (0, 1):
                s = S[b]
                s["lse"] = T(b, "lse")
                nc.scalar.activation(out=s["lse"][:], in_=s["sumexp"][:],
                                     func=AF.Ln)
            for b in (0, 1):
                s = S[b]
                nc.vector.tensor_add(out=s["lse"][:], in0=s["lse"][:],
                                     in1=s["mrow"][:])
                a = s["lse"][:]
                s["lse2"] = bass.AP(tensor=a.tensor, offset=a.offset,
                                    ap=[a.ap[0], [0, 2]])
            # ce(lp) = logaddexp(lp, lse) - lp, batched over [q.k, q.k2]
            for b in (0, 1):
                s = S[b]
                s["mm2"] = T(b, "mm2", w=2)
                nc.vector.tensor_max(out=s["mm2"][:], in0=s["lp2"][:],
                                     in1=s["lse2"])
            for b in (0, 1):
                s = S[b]
                s["mmlp"] = T(b, "mmlp", w=2)
                nc.vector.tensor_sub(out=s["mmlp"][:], in0=s["mm2"][:],
                                     in1=s["lp2"][:])
                s["e12"] = T(b, "e12", w=4)
                nc.vector.tensor_sub(out=s["e12"][:, 0:2], in0=s["lp2"][:],
                                     in1=s["mm2"][:])
                nc.vector.tensor_sub(out=s["e12"][:, 2:4], in0=s["lse2"],
                                     in1=s["mm2"][:])
            for b in (0, 1):
                s = S[b]
                nc.vector.tensor_scalar_max(out=s["e12"][:], in0=s["e12"][:],
                                            scalar1=EXP_CLAMP)
            for b in (0, 1):
                s = S[b]
                nc.scalar.activation(out=s["e12"][:], in_=s["e12"][:],
                                     func=AF.Exp)
            for b in (0, 1):
                s = S[b]
                s["s12"] = T(b, "s12", w=2)
                nc.vector.tensor_add(out=s["s12"][:], in0=s["e12"][:, 0:2],
                                     in1=s["e12"][:, 2:4])
            for b in (0, 1):
                s = S[b]
                nc.scalar.activation(out=s["s12"][:], in_=s["s12"][:],
                                     func=AF.Ln)
            for b in (0, 1):
                s = S[b]
                s["ce2"] = T(b, "ce2", w=2)
                nc.vector.tensor_add(out=s["ce2"][:], in0=s["s12"][:],
                                     in1=s["mmlp"][:])
            # ---- BML term ----
            for b in (0, 1):
                s = S[b]
                s["maskf"] = T(b, "maskf", w=M_MAX)
                nc.vector.tensor_scalar(
                    out=s["maskf"][:], in0=iota_f[:, :M_MAX],
                    scalar1=s["fcf"][:], scalar2=None, op0=OP.is_lt)
                s["simpos"] = T(b, "simpos")
                nc.vector.tensor_scalar_mul(out=s["simpos"][:],
                                            in0=s["lp2"][:, 0:1], scalar1=TEMP)
            for b in (0, 1):
                s = S[b]
                nc.vector.tensor_mul(out=s["fnd"][:], in0=s["fnd"][:],
                                     in1=s["maskf"][:])
            for b in (0, 1):
                s = S[b]
                s["sfn"] = T(b, "sfn")
                nc.vector.tensor_reduce(out=s["sfn"][:], in_=s["fnd"][:],
                                        axis=AX.X, op=OP.add)
                s["den"] = T(b, "den")
                nc.vector.tensor_scalar_max(out=s["den"][:], in0=s["fcf"][:],
                                            scalar1=1.0)
            for b in (0, 1):
                s = S[b]
                s["rden"] = T(b, "rden")
                nc.vector.reciprocal(out=s["rden"][:], in_=s["den"][:])
            for b in (0, 1):
                s = S[b]
                s["simfn"] = T(b, "simfn")
                nc.vector.tensor_mul(out=s["simfn"][:], in0=s["sfn"][:],
                                     in1=s["rden"][:])
            for b in (0, 1):
                s = S[b]
                s["delta"] = T(b, "delta")
                nc.vector.tensor_sub(out=s["delta"][:], in0=s["simfn"][:],
                                     in1=s["simpos"][:])
            for b in (0, 1):
                s = S[b]
                s["r1"] = T(b, "r1")
                nc.scalar.activation(out=s["r1"][:], in_=s["delta"][:],
                                     func=AF.Relu, bias=alpha_t[:], scale=1.0)
                s["r2"] = T(b, "r2")
                nc.scalar.activation(out=s["r2"][:], in_=s["delta"][:],
                                     func=AF.Relu, bias=nbeta_t[:], scale=-1.0)
            for b in (0, 1):
                s = S[b]
                s["bml"] = T(b, "bml")
                nc.vector.tensor_add(out=s["bml"][:], in0=s["r1"][:],
                                     in1=s["r2"][:])
                s["vh"] = T(b, "vh")
                nc.vector.tensor_scalar(out=s["vh"][:], in0=s["hcf"][:],
                                        scalar1=0.0, scalar2=None,
                                        op0=OP.is_gt)
                s["vf"] = T(b, "vf")
                nc.vector.tensor_scalar(out=s["vf"][:], in0=s["fcf"][:],
                                        scalar1=0.0, scalar2=None,
                                        op0=OP.is_gt)
            for b in (0, 1):
                s = S[b]
                s["vb"] = T(b, "vb")
                nc.vector.tensor_mul(out=s["vb"][:], in0=s["vh"][:],
                                     in1=s["vf"][:])
            for b in (0, 1):
                s = S[b]
                contrib = smpool.tile([PBLK, 5], F32, tag=f"contrib{b}",
                                      name=f"contrib{b}")
                nc.vector.tensor_mul(out=contrib[:, 0:1],
                                     in0=s["ce2"][:, 0:1], in1=s["vh"][:])
                nc.vector.tensor_mul(out=contrib[:, 1:2],
                                     in0=s["ce2"][:, 1:2], in1=s["vh"][:])
                nc.vector.tensor_mul(out=contrib[:, 2:3], in0=s["bml"][:],
                                     in1=s["vb"][:])
                nc.vector.tensor_copy(out=contrib[:, 3:4], in_=s["vh"][:])
                nc.vector.tensor_copy(out=contrib[:, 4:5], in_=s["vb"][:])
                blk_contribs[b] = contrib

        for phase in range(2):
            emit_phase(phase)
        hc_f_t, fc_f_t = load_counts()
        for phase in range(2, HALF):
            emit_phase(phase)
        epilogue_both()

        for b in range(2):
            nc.sync.dma_start(out=out[b], in_=blk_contribs[b][:])


def _build(Ns):
    nc = bacc.Bacc("TRN2", target_bir_lowering=False, debug=False)
    total = sum(GRP * 512 * (PRE + N) for N in Ns)
    hp = nc.dram_tensor("hp", [total], BF16, kind="ExternalInput")
    qT = nc.dram_tensor("qT", [PBLK, B_LOC, 4], BF16, kind="ExternalInput")
    hc = nc.dram_tensor("hn_counts", [2, PBLK, 1], I32, kind="ExternalInput")
    fc = nc.dram_tensor("fn_counts", [2, PBLK, 1], I32, kind="ExternalInput")
    out = nc.dram_tensor("out", [2, PBLK, 5], F32, kind="ExternalOutput")
    with tile.TileContext(nc) as tc:
        _emit(tc, Ns, hp, qT, hc, fc, out)
    nc.compile()
    return nc


_NC_CACHE = {}


def _get_nc(key):
    if key not in _NC_CACHE:
        _NC_CACHE[key] = _build(key)
    return _NC_CACHE[key]


def plan(hn_counts):
    """Count-sorted stripe schedule: rank k of core c = order[8k + c].
    Group g (ranks GRP*g .. GRP*g+GRP-1) gets negative bound Ns[g] = max
    count in the group across all cores = sorted_count[8*GRP*g]."""
    counts = np.asarray(hn_counts)
    order = np.argsort(-counts, kind="stable")
    cs = counts[order]
    Ns = tuple(int(cs[N_CORES * GRP * g]) for g in range(NGRP))
    return order, Ns


def make_in_maps(q, k, k2, hn, fn, hn_counts, fn_counts):
    import ml_dtypes
    bf16 = ml_dtypes.bfloat16
    q = np.asarray(q, np.float32)
    k = np.asarray(k, np.float32)
    k2 = np.asarray(k2, np.float32)
    hn = np.asarray(hn, np.float32)
    fn = np.asarray(fn, np.float32)
    hn_counts = np.asarray(hn_counts, np.int32)
    fn_counts = np.asarray(fn_counts, np.int32)

    order, Ns = plan(hn_counts)
    ranks = order.reshape(B_LOC, N_CORES)    # [rank, core]
    sizes = [GRP * 512 * (PRE + N) for N in Ns]
    offs = np.concatenate([[0], np.cumsum(sizes)])
    total = int(offs[-1])

    # pre-transposed per-sample views
    fnT = fn.reshape(B, M_MAX, 4, 128)
    kT = k.reshape(B, 4, 128)
    k2T = k2.reshape(B, 4, 128)
    hnT = hn.reshape(B, N_MAX, 4, 128)

    in_maps = []
    for c in range(N_CORES):
        sc = ranks[:, c]
        hp = np.empty(total, bf16)
        for g in range(NGRP):
            N = Ns[g]
            W = PRE + N
            base = int(offs[g])
            # partition-major group layout: [128, GRP, 4*W]
            hpg = hp[base: base + GRP * 512 * W].reshape(128, GRP, 4 * W)
            for i in range(GRP):
                s = int(sc[GRP * g + i])
                tmp = np.empty((128, 4, W), bf16)
                tmp[:, :, :M_MAX] = fnT[s].transpose(2, 1, 0)
                tmp[:, :, M_MAX] = kT[s].T
                tmp[:, :, M_MAX + 1] = k2T[s].T
                if N:
                    tmp[:, :, PRE:] = hnT[s, :N].transpose(2, 1, 0)
                hpg[:, i, :] = tmp.reshape(128, 4 * W)
        qTc = np.ascontiguousarray(
            q[sc].reshape(B_LOC, 4, 128).transpose(2, 0, 1)).astype(bf16)
        in_maps.append({
            "hp": hp,
            "qT": qTc,
            "hn_counts": hn_counts[sc].reshape(2, PBLK, 1),
            "fn_counts": fn_counts[sc].reshape(2, PBLK, 1),
        })
    return in_maps, Ns


def combine_partials(results):
    parts = np.stack([np.asarray(r["out"], np.float64).sum(axis=(0, 1))
                      for r in results])
    cl_s, clnb_s, bml_s, nv, nb = parts.sum(axis=0)
    n_valid = max(nv, 1.0)
    cl = cl_s / n_valid
    clnb = clnb_s / n_valid
    bml_mean = (bml_s / nb) if nb > 0 else 0.0
    lbml = LAMBDA_BML * bml_mean
    tot = cl + clnb + lbml
    return np.array([tot, cl, lbml, clnb], np.float32)


def run_spmd(in_maps, bounds, **kwargs):
    nc = _get_nc(tuple(bounds))
    return bass_utils.run_bass_kernel_spmd(
        nc, in_maps, core_ids=list(range(N_CORES)), **kwargs
    )


def kernel(q, k, k2, hn, fn, hn_counts, fn_counts):
    in_maps, Ns = make_in_maps(q, k, k2, hn, fn, hn_counts, fn_counts)
    res = run_spmd(in_maps, Ns)
    return combine_partials(res.results)
